# revision 1
# baseline (speedup 1.0000x reference)
"""Trainium2 Bass kernel for JetGNN (2-layer SAGEConv + global mean pool).

Strategy (8 NeuronCores, graph-aligned node sharding):
  - Host: sort nodes per core by degree (desc), build padded per-tile
    neighbor tables (128-node tiles, per-tile max degree K_t, uniform
    across cores), degree/pool index tables.
  - Launch A (SPMD x8): per dst-tile indirect-DMA gather of x[nbr] ->
    strided DVE reduce (neighbor sum) -> mean scale -> PE transpose ->
    matmuls (W1_l aggregate path + W1_r self path, PSUM accumulate) ->
    bias + ReLU -> h1 (bf16) per-core shard.
  - Host: assemble full h1 table (bf16), re-upload.
  - Launch B: same message passing for layer 2 (gather bf16 h1), then
    global pooling via one indirect scatter-add (CCE) into per-graph
    sum buffer. Host applies 1/count and the tiny final linear.
"""

import math
import time

import numpy as np
import ml_dtypes

import concourse.bass as bass
import concourse.tile as tile
import concourse.mybir as mybir
from concourse import bacc
from concourse.bass_utils import run_bass_kernel_spmd
from concourse.masks import make_identity

N_NODES = 200000
N_GRAPHS = 4000
N_CORES = 8
IN_CH = 32
HID = 64
OUT_CH = 2
PAD_N = N_NODES + 64  # zero rows at the tail; ZERO_ROW used for pad neighbors
ZERO_ROW = N_NODES
POOL_SLOTS = 640  # per-core graph slots (~500 graphs/core); last slot = dump
DUMP_SLOT = POOL_SLOTS - 1
P = 128

f32 = mybir.dt.float32
bf16 = mybir.dt.bfloat16
i32 = mybir.dt.int32


# ----------------------------------------------------------------- host prep
def _prep(edge_index, batch):
    """Build per-core padded neighbor tables + layouts. Returns dict."""
    src = np.asarray(edge_index[0], dtype=np.int64)
    dst = np.asarray(edge_index[1], dtype=np.int64)
    batch = np.asarray(batch, dtype=np.int64)
    deg = np.bincount(dst, minlength=N_NODES).astype(np.int64)

    # CSR by dst
    order = np.argsort(dst, kind="stable")
    src_sorted = src[order]
    rowptr = np.zeros(N_NODES + 1, dtype=np.int64)
    np.cumsum(deg, out=rowptr[1:])

    # graph-aligned core boundaries (batch is sorted)
    gcnt = np.bincount(batch, minlength=N_GRAPHS)
    gends = np.cumsum(gcnt)  # node end index per graph
    targets = (np.arange(1, N_CORES) * N_NODES) // N_CORES
    gb = np.searchsorted(gends, targets)  # graph idx whose end >= target
    graph_bounds = np.concatenate([[0], gb + 1, [N_GRAPHS]])
    node_bounds = np.concatenate([[0], gends[graph_bounds[1:-1] - 1], [N_NODES]])

    cores = []
    for c in range(N_CORES):
        lo, hi = node_bounds[c], node_bounds[c + 1]
        ids = np.arange(lo, hi)
        # degree-descending order (stable for determinism)
        perm = np.argsort(-deg[lo:hi], kind="stable")
        ids = ids[perm]
        cores.append(
            dict(ids=ids, glo=graph_bounds[c], ghi=graph_bounds[c + 1]))
        assert graph_bounds[c + 1] - graph_bounds[c] <= DUMP_SLOT

    T = max(math.ceil(len(ci["ids"]) / P) for ci in cores)
    # per-tile K: max over cores of max degree within the tile
    K = np.ones(T, dtype=np.int64)
    for ci in cores:
        d = deg[ci["ids"]]
        d = np.pad(d, (0, T * P - len(d)))
        K = np.maximum(K, d.reshape(T, P).max(axis=1))
    K = K.astype(np.int64)
    offs = np.zeros(T + 1, dtype=np.int64)
    np.cumsum(K, out=offs[1:])
    C = int(offs[-1])

    for ci in cores:
        ids = ci["ids"]
        n = len(ids)
        nbr = np.full((P, C), ZERO_ROW, dtype=np.int32)
        invdeg = np.zeros((P, T), dtype=np.float32)
        localg = np.full((P, T), DUMP_SLOT, dtype=np.int32)
        for t in range(T):
            l0 = t * P
            cnt = min(P, max(0, n - l0))
            if cnt == 0:
                continue
            nodes_t = ids[l0:l0 + cnt]
            invdeg[:cnt, t] = 1.0 / np.maximum(deg[nodes_t], 1).astype(np.float32)
            localg[:cnt, t] = (batch[nodes_t] - ci["glo"]).astype(np.int32)
            kt = int(K[t])
            for p in range(cnt):
                node = nodes_t[p]
                d = int(deg[node])
                if d:
                    nbr[p, offs[t]:offs[t] + d] = src_sorted[
                        rowptr[node]:rowptr[node] + d]
        ci.update(nbr=nbr, invdeg=invdeg, localg=localg, n=n)

    return dict(cores=cores, T=T, K=K.tolist(), offs=offs.tolist(), C=C,
                deg=deg, batch=batch)


# ------------------------------------------------------------ kernel builders
def _build_layer_nc(n_feat, n_hid, T, K, offs, C, pad_n, feat_dtype,
                    self_dtype, pool_slots=None):
    """Build one SAGEConv layer NEFF. If pool_slots: also scatter-add pooling
    (layer-2). Returns (nc, names dict)."""
    nc = bacc.Bacc("TRN2", target_bir_lowering=False, debug=False,
                   enable_asserts=False, num_devices=N_CORES)
    feats = nc.dram_tensor("feats", [pad_n, n_feat], feat_dtype,
                           kind="ExternalInput").ap()
    nbr = nc.dram_tensor("nbr", [P, C], i32, kind="ExternalInput").ap()
    selfT = nc.dram_tensor("selfT", [n_feat, T * P], self_dtype,
                           kind="ExternalInput").ap()
    invdeg = nc.dram_tensor("invdeg", [P, T], f32, kind="ExternalInput").ap()
    wlT = nc.dram_tensor("wlT", [n_feat, n_hid], f32, kind="ExternalInput").ap()
    wrT = nc.dram_tensor("wrT", [n_feat, n_hid], self_dtype,
                         kind="ExternalInput").ap()
    brep = nc.dram_tensor("brep", [P, n_hid], f32, kind="ExternalInput").ap()
    if pool_slots:
        localg = nc.dram_tensor("localg", [P, T], i32,
                                kind="ExternalInput").ap()
        pool = nc.dram_tensor("pool", [pool_slots, n_hid], f32,
                              kind="ExternalOutput").ap()
        out_dt = f32
    else:
        hout = nc.dram_tensor("hout", [P, T * n_hid], bf16,
                              kind="ExternalOutput").ap()
        out_dt = bf16

    with tile.TileContext(nc) as tc:
        with tc.tile_pool(name="resident", bufs=1) as rpool, \
             tc.tile_pool(name="gather", bufs=3) as gpool, \
             tc.tile_pool(name="work", bufs=3) as wpool, \
             tc.tile_pool(name="ps_t", bufs=2, space="PSUM") as ps_t, \
             tc.tile_pool(name="ps_z", bufs=2, space="PSUM") as ps_z:
            nbr_sb = rpool.tile([P, C], i32, tag="nbr")
            nc.sync.dma_start(nbr_sb[:], nbr[:])
            selfT_sb = rpool.tile([n_feat, T * P], self_dtype, tag="selfT")
            nc.sync.dma_start(selfT_sb[:], selfT[:])
            invdeg_sb = rpool.tile([P, T], f32, tag="invdeg")
            nc.sync.dma_start(invdeg_sb[:], invdeg[:])
            wlT_sb = rpool.tile([n_feat, n_hid], f32, tag="wlT")
            nc.sync.dma_start(wlT_sb[:], wlT[:])
            wrT_sb = rpool.tile([n_feat, n_hid], self_dtype, tag="wrT")
            nc.sync.dma_start(wrT_sb[:], wrT[:])
            brep_sb = rpool.tile([P, n_hid], f32, tag="brep")
            nc.sync.dma_start(brep_sb[:], brep[:])
            ident = rpool.tile([P, P], f32, tag="ident")
            make_identity(nc, ident[:])
            if pool_slots:
                localg_sb = rpool.tile([P, T], i32, tag="localg")
                nc.sync.dma_start(localg_sb[:], localg[:])
            hall = rpool.tile([P, T * n_hid], out_dt, tag="hall")

            for t in range(T):
                kt = K[t]
                g = gpool.tile([P, kt * n_feat], feat_dtype, tag="g")
                # HW vector-dynamic-offset DGE is disabled in this env:
                # one indirect DMA per neighbor slot (128 rows, one/partition)
                for k in range(kt):
                    nc.gpsimd.indirect_dma_start(
                        out=g[:, k * n_feat:(k + 1) * n_feat],
                        out_offset=None, in_=feats[:],
                        in_offset=bass.IndirectOffsetOnAxis(
                            ap=nbr_sb[:, offs[t] + k:offs[t] + k + 1],
                            axis=0))
                agg = wpool.tile([P, n_feat], f32, tag="agg")
                nc.vector.tensor_reduce(
                    agg[:],
                    g[:].rearrange("p (k f) -> p f k", k=kt),
                    axis=mybir.AxisListType.X, op=mybir.AluOpType.add)
                nc.vector.tensor_scalar_mul(agg[:], agg[:],
                                            invdeg_sb[:, t:t + 1])
                aggT_ps = ps_t.tile([n_feat, P], f32, tag="aggT")
                nc.tensor.transpose(aggT_ps[:], agg[:], ident[:])
                aggT = wpool.tile([n_feat, P], f32, tag="aggTs")
                nc.vector.tensor_copy(aggT[:], aggT_ps[:])
                z_ps = ps_z.tile([P, n_hid], f32, tag="z")
                nc.tensor.matmul(z_ps[:], lhsT=aggT[:], rhs=wlT_sb[:],
                                 start=True, stop=False)
                nc.tensor.matmul(z_ps[:], lhsT=selfT_sb[:, t * P:(t + 1) * P],
                                 rhs=wrT_sb[:], start=False, stop=True)
                zb = wpool.tile([P, n_hid], f32, tag="zb")
                nc.vector.tensor_tensor(zb[:], z_ps[:], brep_sb[:],
                                        op=mybir.AluOpType.add)
                nc.scalar.activation(hall[:, t * n_hid:(t + 1) * n_hid],
                                     zb[:], mybir.ActivationFunctionType.Relu)

            if pool_slots:
                nc.gpsimd.indirect_dma_start(
                    out=pool[:],
                    out_offset=bass.IndirectOffsetOnAxis(ap=localg_sb[:],
                                                         axis=0),
                    in_=hall[:], in_offset=None,
                    compute_op=mybir.AluOpType.add)
            else:
                nc.sync.dma_start(hout[:], hall[:])

    nc.compile()
    return nc


# ---------------------------------------------------------------- run helper
def _run(nc, in_maps):
    t0 = time.time()
    res = run_bass_kernel_spmd(nc, in_maps, core_ids=list(range(N_CORES)))
    dt = time.time() - t0
    return res.results, dt


def kernel(x, edge_index, batch, W1_l, b1, W1_r, W2_l, b2, W2_r, W_lin,
           b_lin, _timing=None):
    x = np.asarray(x, dtype=np.float32)
    W1_l = np.asarray(W1_l, np.float32)
    W1_r = np.asarray(W1_r, np.float32)
    W2_l = np.asarray(W2_l, np.float32)
    W2_r = np.asarray(W2_r, np.float32)
    b1 = np.asarray(b1, np.float32)
    b2 = np.asarray(b2, np.float32)
    W_lin = np.asarray(W_lin, np.float32)
    b_lin = np.asarray(b_lin, np.float32)

    pp = _prep(edge_index, batch)
    T, K, offs, C = pp["T"], pp["K"], pp["offs"], pp["C"]

    x_pad = np.zeros((PAD_N, IN_CH), np.float32)
    x_pad[:N_NODES] = x

    # ---- launch A (layer 1)
    ncA = _build_layer_nc(IN_CH, HID, T, K, offs, C, PAD_N, f32, f32)
    b1rep = np.broadcast_to(b1, (P, HID)).copy()
    in_maps = []
    for ci in pp["cores"]:
        xs = np.zeros((T * P, IN_CH), np.float32)
        xs[:ci["n"]] = x[ci["ids"]]
        in_maps.append(dict(
            feats=x_pad, nbr=ci["nbr"], selfT=np.ascontiguousarray(xs.T),
            invdeg=ci["invdeg"], wlT=np.ascontiguousarray(W1_l.T),
            wrT=np.ascontiguousarray(W1_r.T), brep=b1rep))
    resA, dtA = _run(ncA, in_maps)

    # ---- host: assemble full h1 (bf16)
    h1_full = np.zeros((PAD_N, HID), ml_dtypes.bfloat16)
    for ci, r in zip(pp["cores"], resA):
        ht = r["hout"].reshape(P, T, HID).transpose(1, 0, 2).reshape(
            T * P, HID)
        h1_full[ci["ids"]] = ht[:ci["n"]]

    kernel._dbg = dict(h1_full=h1_full, pp=pp, resA=resA)

    # ---- launch B (layer 2)
    ncB = _build_layer_nc(HID, HID, T, K, offs, C, PAD_N, bf16, bf16)
    b2rep = np.broadcast_to(b2, (P, HID)).copy()
    in_maps = []
    for ci in pp["cores"]:
        hs = np.zeros((T * P, HID), ml_dtypes.bfloat16)
        hs[:ci["n"]] = h1_full[ci["ids"]]
        in_maps.append(dict(
            feats=h1_full, nbr=ci["nbr"], selfT=np.ascontiguousarray(hs.T),
            invdeg=ci["invdeg"],
            wlT=np.ascontiguousarray(W2_l.T),
            wrT=np.ascontiguousarray(W2_r.T).astype(ml_dtypes.bfloat16),
            brep=b2rep))
    resB, dtB = _run(ncB, in_maps)

    # ---- host: global mean pool + final linear (0.3% of model FLOPs)
    pool = np.zeros((N_GRAPHS, HID), np.float32)
    for ci, r in zip(pp["cores"], resB):
        ht = r["hout"].reshape(P, T, HID).transpose(1, 0, 2).reshape(
            T * P, HID)[:ci["n"]].astype(np.float32)
        np.add.at(pool, pp["batch"][ci["ids"]], ht)
    gcnt = np.bincount(pp["batch"], minlength=N_GRAPHS).astype(np.float32)
    pooled = pool / np.maximum(gcnt, 1.0)[:, None]
    out = (pooled @ W_lin.T + b_lin).astype(np.float32)
    if _timing is not None:
        _timing.update(dtA=dtA, dtB=dtB)
    return out



# revision 4
# speedup vs baseline: 6.4298x; 6.4298x over previous
"""Trainium2 Bass kernel for JetGNN (2-layer SAGEConv + global mean pool).

Single fused NEFF, src-major sharding:
  - Host: graph-aligned 25600-node slabs per core; each core owns the edges
    whose SRC lies in its slab. Edges grouped by (dst supertile of 512
    global dst slots, src%4), padded to 128-edge chunks.
  - Device per layer: For_i over 400 supertiles: dma_gather of packed fp16
    feature rows (4 nodes/row for x, 2 for h1), one-hot matmuls
    (is_equal(iota, dstv) * invdeg) accumulate mean-message partials
    [F, 512] in PSUM -> fp16 partials [F, 204800] -> AllReduce(add) ->
    per-own-tile dst phase: W_l @ agg + W_r @ self, bias+ReLU; h1 kept
    resident (self path) and written packed to DRAM (layer-2 gather).
  - Host: global mean pool (cumsum diff over sorted batch) + final linear.
"""

import math
import time

import numpy as np
import ml_dtypes

import concourse.bass as bass
import concourse.tile as tile
import concourse.mybir as mybir
from concourse import bacc
from concourse.bass_utils import run_bass_kernel_spmd

N_NODES = 200000
N_GRAPHS = 4000
N_CORES = 8
IN_CH = 32
HID = 64
OUT_CH = 2
SLAB = 25600
NSLOT = N_CORES * SLAB          # 204800 global dst slots
ST = 512                        # dsts per supertile
NS = NSLOT // ST                # 400 supertiles
P = 128
PAD_DSTV = 600.0                # one-hot never matches

f32 = mybir.dt.float32
fp16 = mybir.dt.float16
i16 = mybir.dt.int16

GRP = [[0, 1, 2, 3, 4, 5, 6, 7]]


# ----------------------------------------------------------------- host prep
def _prep(edge_index, batch):
    src = np.asarray(edge_index[0], dtype=np.int64)
    dst = np.asarray(edge_index[1], dtype=np.int64)
    batch = np.asarray(batch, dtype=np.int64)

    gcnt = np.bincount(batch, minlength=N_GRAPHS)
    gends = np.cumsum(gcnt)
    targets = (np.arange(1, N_CORES) * N_NODES) // N_CORES
    gb = np.searchsorted(gends, targets)
    graph_bounds = np.concatenate([[0], gb + 1, [N_GRAPHS]])
    node_bounds = np.concatenate(
        [[0], gends[graph_bounds[1:-1] - 1], [N_NODES]]).astype(np.int64)
    ncounts = np.diff(node_bounds)
    assert ncounts.max() <= SLAB, ncounts.max()

    deg = np.bincount(dst, minlength=N_NODES)
    inv = (1.0 / np.maximum(deg, 1)).astype(np.float32)

    cs = np.searchsorted(node_bounds, src, side="right") - 1
    cd = np.searchsorted(node_bounds, dst, side="right") - 1
    src_local = src - node_bounds[cs]
    dslot = cd * SLAB + (dst - node_bounds[cd])
    s_id = dslot >> 9
    din = (dslot & 511).astype(np.float32)
    par = (src_local & 3).astype(np.int64)

    # group edges by (core, supertile, parity)
    key = ((cs * NS + s_id) << 2) | par
    order = np.argsort(key, kind="stable")
    key_s = key[order]
    nbins = N_CORES * NS * 4
    cnt = np.bincount(key_s, minlength=nbins)
    ch_par = max(1, int(math.ceil(cnt.max() / P)))      # chunks per parity
    nch = 4 * ch_par                                     # chunks / supertile
    slots_st = nch * P
    # gathers per supertile: groups of <= 8 chunks
    ngath = int(math.ceil(nch / 8))
    gsz = int(math.ceil(nch / ngath))                    # chunks per gather
    nch_pad = ngath * gsz
    slots_pad = nch_pad * P

    starts = np.concatenate([[0], np.cumsum(cnt)[:-1]])
    rank = np.arange(len(src)) - starts[key_s]           # pos within group

    # slot of each (sorted) edge in its core's [NS, 4, ch_par*128] layout
    k = key_s
    core_e = k // (NS * 4)
    s_e = (k // 4) % NS
    p_e = k & 3
    slot = ((core_e * NS + s_e) * nch_pad + p_e * ch_par) * P + rank

    total = N_CORES * NS * slots_pad
    idx1 = np.zeros(total, np.int16)
    idx2 = np.zeros(total, np.int16)
    dstv = np.full(total, PAD_DSTV, np.float16)
    invv = np.zeros(total, np.float16)
    sl = src_local[order]
    idx1[slot] = (sl >> 2).astype(np.int16)
    idx2[slot] = (sl >> 1).astype(np.int16)
    dstv[slot] = din[order].astype(np.float16)
    invv[slot] = inv[dst[order]].astype(np.float16)

    # idx compact wrap: [core][16, NS * slots_pad/16]
    def wrapidx(a):
        a = a.reshape(N_CORES, NS, ngath, gsz * 8, 16)
        a = a.transpose(0, 4, 1, 2, 3)                   # [NC,16,NS,ngath,gsz*8]
        return np.ascontiguousarray(a.reshape(N_CORES, 16, -1))

    idx1w = wrapidx(idx1)
    idx2w = wrapidx(idx2)

    # dv table: [core][128, NS * 2*nch] fp16 : per supertile nch dstv cols
    # then nch inv cols; value at (partition slot%128, col chunk)
    dstv = dstv.reshape(N_CORES, NS, nch_pad, P)
    invv = invv.reshape(N_CORES, NS, nch_pad, P)
    dv = np.concatenate([dstv, invv], axis=2)            # [NC, NS, 2nch, P]
    dv = np.ascontiguousarray(
        dv.transpose(0, 3, 1, 2).reshape(N_CORES, P, -1))

    return dict(node_bounds=node_bounds, graph_bounds=graph_bounds,
                ncounts=ncounts, gcnt=gcnt, ch_par=ch_par, nch=nch_pad,
                ngath=ngath, gsz=gsz, idx1w=idx1w, idx2w=idx2w, dv=dv)


# ------------------------------------------------------------ kernel builder
def _build_nc(nch, ngath, gsz, ch_par):
    icols = NS * ngath * gsz * 8          # idx cols per 16-partition row
    ic_st = ngath * gsz * 8               # idx cols per supertile
    dcols = 2 * nch

    nc = bacc.Bacc("TRN2", target_bir_lowering=False, debug=False,
                   enable_asserts=False, num_devices=N_CORES)
    xg = nc.dram_tensor("xg", [SLAB * IN_CH // P, P], fp16,
                        kind="ExternalInput").ap()
    selfxT_d = nc.dram_tensor("selfxT", [IN_CH, SLAB], fp16,
                              kind="ExternalInput").ap()
    idx1c = nc.dram_tensor("idx1c", [16, icols], i16, kind="ExternalInput").ap()
    idx2c = nc.dram_tensor("idx2c", [16, icols], i16, kind="ExternalInput").ap()
    dvd = nc.dram_tensor("dvd", [P, NS * dcols], fp16,
                         kind="ExternalInput").ap()
    iota_d = nc.dram_tensor("iota", [P, ST], fp16, kind="ExternalInput").ap()
    ident_d = nc.dram_tensor("ident", [HID, HID], fp16,
                             kind="ExternalInput").ap()
    w1lT_d = nc.dram_tensor("w1lT", [IN_CH, HID], fp16,
                            kind="ExternalInput").ap()
    w1rT_d = nc.dram_tensor("w1rT", [IN_CH, HID], fp16,
                            kind="ExternalInput").ap()
    w2lT_d = nc.dram_tensor("w2lT", [HID, HID], fp16,
                            kind="ExternalInput").ap()
    w2rT_d = nc.dram_tensor("w2rT", [HID, HID], fp16,
                            kind="ExternalInput").ap()
    b1_d = nc.dram_tensor("b1c", [HID, 1], f32, kind="ExternalInput").ap()
    b2_d = nc.dram_tensor("b2c", [HID, 1], f32, kind="ExternalInput").ap()

    idx1r = nc.dram_tensor("idx1r", [P, icols], i16, kind="Internal").ap()
    idx2r = nc.dram_tensor("idx2r", [P, icols], i16, kind="Internal").ap()
    h1d = nc.dram_tensor("h1d", [SLAB, HID], fp16, kind="Internal").ap()
    part1 = nc.dram_tensor("part1", [IN_CH, NSLOT], fp16, kind="Internal").ap()
    part2 = nc.dram_tensor("part2", [HID, NSLOT], fp16, kind="Internal").ap()
    red1 = nc.dram_tensor("red1", [IN_CH, NSLOT], fp16, kind="Internal").ap()
    red2 = nc.dram_tensor("red2", [HID, NSLOT], fp16, kind="Internal").ap()
    h2_out = nc.dram_tensor("h2", [SLAB, HID], fp16, kind="ExternalOutput").ap()

    NT = SLAB // P                        # own dst tiles (200)

    with tile.TileContext(nc) as tc:
        with tc.tile_pool(name="res", bufs=1) as rp, \
             tc.tile_pool(name="ld", bufs=3) as ld, \
             tc.tile_pool(name="g", bufs=3) as gp, \
             tc.tile_pool(name="oh", bufs=4) as ohp, \
             tc.tile_pool(name="o", bufs=3) as op_, \
             tc.tile_pool(name="st", bufs=3) as stp, \
             tc.tile_pool(name="ps", bufs=2, space="PSUM") as ps, \
             tc.tile_pool(name="ps2", bufs=2, space="PSUM") as ps2:

            # ---- prologue: residents + idx replication to 128 partitions
            iota_sb = rp.tile([P, ST], fp16, tag="iota")
            nc.sync.dma_start(iota_sb[:], iota_d[:])
            ident_sb = rp.tile([HID, HID], fp16, tag="ident")
            nc.sync.dma_start(ident_sb[:], ident_d[:])
            w1lT = rp.tile([IN_CH, HID], fp16, tag="w1lT")
            nc.sync.dma_start(w1lT[:], w1lT_d[:])
            w1rT = rp.tile([IN_CH, HID], fp16, tag="w1rT")
            nc.sync.dma_start(w1rT[:], w1rT_d[:])
            w2lT = rp.tile([HID, HID], fp16, tag="w2lT")
            nc.sync.dma_start(w2lT[:], w2lT_d[:])
            w2rT = rp.tile([HID, HID], fp16, tag="w2rT")
            nc.sync.dma_start(w2rT[:], w2rT_d[:])
            b1 = rp.tile([HID, 1], f32, tag="b1")
            nc.sync.dma_start(b1[:], b1_d[:])
            b2 = rp.tile([HID, 1], f32, tag="b2")
            nc.sync.dma_start(b2[:], b2_d[:])
            selfxT = rp.tile([IN_CH, SLAB], fp16, tag="selfxT")
            nc.sync.dma_start(selfxT[:], selfxT_d[:])
            h1T_res = rp.tile([HID, SLAB], fp16, tag="h1T_res")
            red_sb = rp.tile([HID, SLAB], fp16, tag="red_sb")

            for k in range(8):
                nc.sync.dma_start(idx1r[16 * k:16 * (k + 1), :], idx1c[:])
                nc.sync.dma_start(idx2r[16 * k:16 * (k + 1), :], idx2c[:])
            tc.strict_bb_all_engine_barrier()

            pid = nc.sync.partition_id()

            def supertile_loop(lay, idxr, tabv, F, partials):
                idx3 = idxr.rearrange("p (s c) -> p s c", c=ic_st)
                dv3 = dvd.rearrange("p (s c) -> p s c", c=dcols)
                p3 = partials.rearrange("f (s d) -> f s d", d=ST)
                gslots = gsz * P
                with tc.For_i(0, NS) as s:
                    idx_sb = ld.tile([P, ic_st], i16, tag=f"idx{lay}")
                    nc.sync.dma_start(idx_sb[:], idx3[:, s])
                    dvh = ld.tile([P, dcols], fp16, tag=f"dvh{lay}")
                    nc.sync.dma_start(dvh[:], dv3[:, s])
                    dvf = ld.tile([P, dcols], f32, tag=f"dvf{lay}")
                    nc.vector.tensor_copy(dvf[:], dvh[:])
                    ms = []
                    for g in range(ngath):
                        m = gp.tile([P, gsz, P], fp16, tag=f"m{lay}_{g}")
                        nc.gpsimd.dma_gather(
                            m[:], tabv,
                            idx_sb[:, g * gsz * 8:(g + 1) * gsz * 8],
                            gslots, gslots, P)
                        ms.append(m)
                    zp = ps.tile([F, ST], f32, tag="zp")
                    for c in range(nch):
                        par4 = min(c // ch_par, 3)
                        colblk = (par4 * IN_CH) if lay == 1 else \
                            ((par4 & 1) * HID)
                        m = ms[c // gsz]
                        oh = ohp.tile([P, ST], fp16, tag=f"oh{lay}")
                        nc.vector.tensor_scalar(
                            oh[:], iota_sb[:], dvf[:, c:c + 1],
                            dvf[:, nch + c:nch + c + 1],
                            op0=mybir.AluOpType.is_equal,
                            op1=mybir.AluOpType.mult)
                        nc.tensor.matmul(
                            zp[:], lhsT=m[:, c % gsz, colblk:colblk + F],
                            rhs=oh[:], start=(c == 0), stop=(c == nch - 1))
                    zsb = op_.tile([F, ST], fp16, tag=f"zsb{lay}")
                    nc.vector.tensor_copy(zsb[:], zp[:])
                    nc.sync.dma_start(p3[:, s], zsb[:])

            def dst_loop(lay, red, F, wl, wr, bb, self_sb, hout_res):
                redv = red_sb[0:F, :]
                nc.sync.dma_start(
                    redv,
                    red.rearrange("f (c n) -> f c n", n=SLAB)[:, pid])
                r3 = redv.rearrange("f (t d) -> f t d", d=P)
                s3 = self_sb.rearrange("f (t d) -> f t d", d=P)
                o3 = (hout_res.rearrange("f (t d) -> f t d", d=P)
                      if hout_res is not None else None)
                out_d = h1d if lay == 1 else h2_out
                od3 = out_d.rearrange("(t d) f -> t d f", d=P)
                with tc.For_i(0, NT) as t:
                    z2 = ps2.tile([HID, P], f32, tag="z2")
                    nc.tensor.matmul(z2[:], lhsT=wl[:], rhs=r3[:, t],
                                     start=True, stop=False)
                    nc.tensor.matmul(z2[:], lhsT=wr[:], rhs=s3[:, t],
                                     start=False, stop=True)
                    hT = op_.tile([HID, P], fp16, tag=f"hT{lay}")
                    nc.scalar.activation(hT[:], z2[:],
                                         mybir.ActivationFunctionType.Relu,
                                         bias=bb[:])
                    if o3 is not None:
                        nc.vector.tensor_copy(o3[:, t], hT[:])
                    tp = ps2.tile([P, HID], fp16, tag="tp")
                    nc.tensor.transpose(tp[:], hT[:], ident_sb[:])
                    stg = stp.tile([P, HID], fp16, tag=f"stg{lay}")
                    nc.vector.tensor_copy(stg[:], tp[:])
                    nc.sync.dma_start(od3[t], stg[:])

            # ---- layer 1
            xgv = xg[:]
            supertile_loop(1, idx1r, xgv, IN_CH, part1)
            nc.gpsimd.collective_compute(
                kind="AllReduce", op=mybir.AluOpType.add, replica_groups=GRP,
                ins=[part1[:]], outs=[red1[:]])
            dst_loop(1, red1, IN_CH, w1lT, w1rT, b1, selfxT, h1T_res)

            # ---- layer 2
            h1v = h1d.rearrange("(r k) f -> r (k f)", k=2)
            supertile_loop(2, idx2r, h1v, HID, part2)
            nc.gpsimd.collective_compute(
                kind="AllReduce", op=mybir.AluOpType.add, replica_groups=GRP,
                ins=[part2[:]], outs=[red2[:]])
            dst_loop(2, red2, HID, w2lT, w2rT, b2, h1T_res, None)

    nc.compile()
    return nc


_NC_CACHE = {}


def kernel(x, edge_index, batch, W1_l, b1, W1_r, W2_l, b2, W2_r, W_lin,
           b_lin, _timing=None):
    x = np.asarray(x, dtype=np.float32)
    batch_np = np.asarray(batch, dtype=np.int64)

    t0 = time.time()
    pp = _prep(edge_index, batch_np)
    t_prep = time.time() - t0

    nch, ngath, gsz = pp["nch"], pp["ngath"], pp["gsz"]
    t0 = time.time()
    key = (nch, ngath, gsz, pp["ch_par"])
    if key not in _NC_CACHE:
        _NC_CACHE[key] = _build_nc(nch, ngath, gsz, pp["ch_par"])
    nc = _NC_CACHE[key]
    t_build = time.time() - t0

    nb = pp["node_bounds"]
    iota_np = np.tile(np.arange(ST, dtype=np.float16), (P, 1))
    ident_np = np.eye(HID, dtype=np.float16)
    com = dict(
        iota=iota_np, ident=ident_np,
        w1lT=np.ascontiguousarray(np.asarray(W1_l).T).astype(np.float16),
        w1rT=np.ascontiguousarray(np.asarray(W1_r).T).astype(np.float16),
        w2lT=np.ascontiguousarray(np.asarray(W2_l).T).astype(np.float16),
        w2rT=np.ascontiguousarray(np.asarray(W2_r).T).astype(np.float16),
        b1c=np.asarray(b1, np.float32).reshape(HID, 1),
        b2c=np.asarray(b2, np.float32).reshape(HID, 1),
    )
    in_maps = []
    for c in range(N_CORES):
        lo, hi = nb[c], nb[c + 1]
        xs = np.zeros((SLAB, IN_CH), np.float16)
        xs[:hi - lo] = x[lo:hi].astype(np.float16)
        in_maps.append(dict(
            xg=np.ascontiguousarray(xs.reshape(SLAB * IN_CH // P, P)),
            selfxT=np.ascontiguousarray(xs.T),
            idx1c=pp["idx1w"][c], idx2c=pp["idx2w"][c], dvd=pp["dv"][c],
            **com))

    t0 = time.time()
    res = run_bass_kernel_spmd(nc, in_maps, core_ids=list(range(N_CORES)))
    t_run = time.time() - t0

    # ---- host: global mean pool + final linear
    t0 = time.time()
    pool = np.zeros((N_GRAPHS, HID), np.float32)
    gb = pp["graph_bounds"]
    for c in range(N_CORES):
        n_c = pp["ncounts"][c]
        h2 = res.results[c]["h2"][:n_c].astype(np.float32)
        cs = np.cumsum(h2, axis=0)
        glo, ghi = gb[c], gb[c + 1]
        ends = np.cumsum(pp["gcnt"][glo:ghi])
        sums = cs[ends - 1]
        sums[1:] -= cs[ends[:-1] - 1]
        # graphs with zero nodes contribute zero
        nz = pp["gcnt"][glo:ghi] > 0
        pool[glo:ghi][nz] = sums[nz]
    cnt = np.maximum(pp["gcnt"], 1).astype(np.float32)
    pooled = pool / cnt[:, None]
    out = (pooled @ np.asarray(W_lin, np.float32).T
           + np.asarray(b_lin, np.float32)).astype(np.float32)
    t_host = time.time() - t0

    if _timing is not None:
        _timing.update(dtA=t_run, dtB=0.0, prep=t_prep, build=t_build,
                       host=t_host)
    return out


# revision 14
# speedup vs baseline: 9.9763x; 1.5516x over previous
"""Trainium2 Bass kernel for JetGNN (2-layer SAGEConv + global mean pool).

Single fused NEFF, src-major sharding:
  - Host: graph-aligned 25600-node slabs per core; each core owns the edges
    whose SRC lies in its slab. Edges grouped by (dst supertile of 512
    global dst slots, src%4), padded to 128-edge chunks.
  - Device per layer: For_i over 400 supertiles: dma_gather of packed fp16
    feature rows (4 nodes/row for x, 2 for h1), one-hot matmuls
    (is_equal(iota, dstv)) accumulate message partials [F, 512] in PSUM,
    scaled by per-dst 1/deg row -> fp16 partials [F, 204800] ->
    AllReduce(add) -> per-own-tile dst phase: W_l @ agg + W_r @ self,
    bias+ReLU; h1 kept resident (self path) and written packed to DRAM
    (layer-2 gather). Layer-2 idx derived on device (2*idx1 + gather half).
    Pooling on device: per-tile one-hot matmul into [16,64] graph windows.
  - Host: combine pool windows, divide by counts, final linear.
"""

import math
import os
import threading
import time

import numpy as np

import jax

try:
    jax.config.update("jax_compilation_cache_dir",
                      "/root/.cache/jax_bass_cache")
    jax.config.update("jax_persistent_cache_min_compile_time_secs", 0)
    jax.config.update("jax_persistent_cache_min_entry_size_bytes", 0)
except Exception:
    pass

import concourse.bass as bass
import concourse.tile as tile
import concourse.mybir as mybir
from concourse import bacc
from concourse.bass_utils import run_bass_kernel_spmd

N_NODES = 200000
N_GRAPHS = 4000
N_CORES = 8
IN_CH = 32
HID = 64
SLAB = 25600
NSLOT = N_CORES * SLAB          # 204800 global dst slots
ST = 512                        # dsts per supertile
NS = NSLOT // ST                # 400 supertiles
P = 128
NT = SLAB // P                  # own dst tiles (200)
GT = 16                         # graph window per tile (pooling)
PAD_DSTV = 600.0                # one-hot never matches

f32 = mybir.dt.float32
fp16 = mybir.dt.float16
i16 = mybir.dt.int16

GRP = [[0, 1, 2, 3, 4, 5, 6, 7]]


# ----------------------------------------------------------------- host prep
def _prep(edge_index, batch):
    src = np.asarray(edge_index[0], dtype=np.int32)
    dst = np.asarray(edge_index[1], dtype=np.int32)
    batch = np.asarray(batch, dtype=np.int32)

    gcnt = np.bincount(batch, minlength=N_GRAPHS)
    gends = np.cumsum(gcnt)
    targets = (np.arange(1, N_CORES) * N_NODES) // N_CORES
    gb = np.searchsorted(gends, targets)
    graph_bounds = np.concatenate([[0], gb + 1, [N_GRAPHS]])
    node_bounds = np.concatenate(
        [[0], gends[graph_bounds[1:-1] - 1], [N_NODES]]).astype(np.int64)
    ncounts = np.diff(node_bounds)
    assert ncounts.max() <= SLAB, ncounts.max()

    deg = np.bincount(dst, minlength=N_NODES)
    inv = (1.0 / np.maximum(deg, 1)).astype(np.float32)

    # node -> (core, local, slot) lookup tables
    node_core = np.repeat(np.arange(N_CORES, dtype=np.int32), ncounts)
    node_local = (np.arange(N_NODES, dtype=np.int32)
                  - np.repeat(node_bounds[:-1].astype(np.int32), ncounts))
    node_slot = node_core * SLAB + node_local

    cs = node_core[src]
    src_local = node_local[src]
    dslot = node_slot[dst]
    s_id = dslot >> 9
    din = (dslot & 511).astype(np.float32)
    par = src_local & 3

    # per-dst-slot inverse degree row [NS, 512]
    invrow = np.zeros(NSLOT, np.float16)
    for c in range(N_CORES):
        lo, hi = node_bounds[c], node_bounds[c + 1]
        invrow[c * SLAB:c * SLAB + hi - lo] = inv[lo:hi]
    invrow = invrow.reshape(NS, ST)

    # group edges by (core, supertile, parity)
    key = (((cs * NS + s_id) << 2) | par).astype(np.int16)
    order = np.argsort(key, kind="stable")
    key_s = key[order].astype(np.int32)
    nbins = N_CORES * NS * 4
    cnt = np.bincount(key_s, minlength=nbins)
    ch_par = max(1, int(math.ceil(cnt.max() / P)))      # chunks per parity
    nch = 4 * ch_par                                     # chunks / supertile
    ngath = 2
    gsz = 2 * ch_par                                     # chunks per gather
    assert gsz * P <= 1024, gsz
    slots_pad = nch * P

    starts = np.concatenate([[0], np.cumsum(cnt)[:-1]]).astype(np.int64)
    rank = np.arange(len(src), dtype=np.int64) - starts[key_s]

    k = key_s.astype(np.int64)
    core_e = k // (NS * 4)
    s_e = (k // 4) % NS
    p_e = k & 3
    slot = ((core_e * NS + s_e) * nch + p_e * ch_par) * P + rank

    total = N_CORES * NS * slots_pad
    idx1 = np.zeros(total, np.int16)
    dstv = np.full(total, PAD_DSTV, np.float16)
    sl = src_local[order]
    idx1[slot] = (sl >> 2).astype(np.int16)
    dstv[slot] = din[order].astype(np.float16)

    # idx compact wrap: [core][16, NS * slots_pad/16]
    a = idx1.reshape(N_CORES, NS, ngath, gsz * 8, 16)
    idx1w = np.ascontiguousarray(
        a.transpose(0, 4, 1, 2, 3).reshape(N_CORES, 16, -1))

    # dstv table: [core][128, NS * nch] fp16, value at (slot%128, chunk col)
    dstv = dstv.reshape(N_CORES, NS, nch, P)
    dv = np.ascontiguousarray(
        dstv.transpose(0, 3, 1, 2).reshape(N_CORES, P, -1))

    # pooling tables: g0 per (core, tile); grel [core][128, NT] f32
    g0 = np.zeros((N_CORES, NT), np.int64)
    grel = np.full((N_CORES, P, NT), 100.0, np.float32)
    for c in range(N_CORES):
        lo, hi = node_bounds[c], node_bounds[c + 1]
        bl = batch[lo:hi]
        for t in range((hi - lo + P - 1) // P):
            seg = bl[t * P:(t + 1) * P]
            g0[c, t] = seg[0]
            r = seg - seg[0]
            assert r.max() < GT, r.max()
            grel[c, :len(seg), t] = r
    return dict(node_bounds=node_bounds, graph_bounds=graph_bounds,
                ncounts=ncounts, gcnt=gcnt, ch_par=ch_par, nch=nch,
                ngath=ngath, gsz=gsz, idx1w=idx1w, dv=dv, invrow=invrow,
                g0=g0, grel=grel)


# ------------------------------------------------------------ kernel builder
def _build_nc(nch, ngath, gsz, ch_par):
    icols = NS * ngath * gsz * 8          # idx cols per 16-partition row
    ic_st = ngath * gsz * 8               # idx cols per supertile
    half = gsz * 8                        # idx cols per gather

    nc = bacc.Bacc("TRN2", target_bir_lowering=False, debug=False,
                   enable_asserts=False, num_devices=N_CORES)
    xg = nc.dram_tensor("xg", [SLAB * IN_CH // P, P], fp16,
                        kind="ExternalInput").ap()
    idx1c = nc.dram_tensor("idx1c", [16, icols], i16, kind="ExternalInput").ap()
    dvd = nc.dram_tensor("dvd", [P, NS * nch], fp16,
                         kind="ExternalInput").ap()
    invd = nc.dram_tensor("invd", [NS, ST], fp16, kind="ExternalInput").ap()
    iota_d = nc.dram_tensor("iota", [P, ST], fp16, kind="ExternalInput").ap()
    ident_d = nc.dram_tensor("ident", [P, P], fp16, kind="ExternalInput").ap()
    grel_d = nc.dram_tensor("grel", [P, NT], f32, kind="ExternalInput").ap()
    w1lT_d = nc.dram_tensor("w1lT", [IN_CH, HID], fp16,
                            kind="ExternalInput").ap()
    w1rT_d = nc.dram_tensor("w1rT", [IN_CH, HID], fp16,
                            kind="ExternalInput").ap()
    w2lT_d = nc.dram_tensor("w2lT", [HID, HID], fp16,
                            kind="ExternalInput").ap()
    w2rT_d = nc.dram_tensor("w2rT", [HID, HID], fp16,
                            kind="ExternalInput").ap()
    b1_d = nc.dram_tensor("b1c", [HID, 1], f32, kind="ExternalInput").ap()
    b2_d = nc.dram_tensor("b2c", [HID, 1], f32, kind="ExternalInput").ap()

    idx1r = nc.dram_tensor("idx1r", [P, icols], i16, kind="Internal").ap()
    h1d = nc.dram_tensor("h1d", [SLAB, HID], fp16, kind="Internal").ap()
    part1 = nc.dram_tensor("part1", [IN_CH, NSLOT], fp16, kind="Internal").ap()
    part2 = nc.dram_tensor("part2", [HID, NSLOT], fp16, kind="Internal").ap()
    red1 = nc.dram_tensor("red1", [IN_CH, NSLOT], fp16, kind="Internal").ap()
    red2 = nc.dram_tensor("red2", [HID, NSLOT], fp16, kind="Internal").ap()
    poolp = nc.dram_tensor("poolp", [NT, GT, HID], fp16,
                           kind="ExternalOutput").ap()

    ns_run = int(os.environ.get("K_NS", NS))
    nt_run = int(os.environ.get("K_NT", NT))

    with tile.TileContext(nc) as tc:
        with tc.tile_pool(name="res", bufs=1) as rp, \
             tc.tile_pool(name="ld", bufs=3) as ld, \
             tc.tile_pool(name="g", bufs=3) as gp, \
             tc.tile_pool(name="oh", bufs=4) as ohp, \
             tc.tile_pool(name="o", bufs=3) as op_, \
             tc.tile_pool(name="st", bufs=3) as stp, \
             tc.tile_pool(name="ps", bufs=2, space="PSUM") as ps, \
             tc.tile_pool(name="ps2", bufs=2, space="PSUM") as ps2, \
             tc.tile_pool(name="ps3", bufs=1, space="PSUM") as ps3:

            # ---- prologue: residents + idx replication to 128 partitions
            iota_sb = rp.tile([P, ST], fp16, tag="iota")
            nc.sync.dma_start(iota_sb[:], iota_d[:])
            ident_sb = rp.tile([P, P], fp16, tag="ident")
            nc.sync.dma_start(ident_sb[:], ident_d[:])
            grel_sb = rp.tile([P, NT], f32, tag="grel")
            nc.sync.dma_start(grel_sb[:], grel_d[:])
            w1lT = rp.tile([IN_CH, HID], fp16, tag="w1lT")
            nc.sync.dma_start(w1lT[:], w1lT_d[:])
            w1rT = rp.tile([IN_CH, HID], fp16, tag="w1rT")
            nc.sync.dma_start(w1rT[:], w1rT_d[:])
            w2lT = rp.tile([HID, HID], fp16, tag="w2lT")
            nc.sync.dma_start(w2lT[:], w2lT_d[:])
            w2rT = rp.tile([HID, HID], fp16, tag="w2rT")
            nc.sync.dma_start(w2rT[:], w2rT_d[:])
            b1 = rp.tile([HID, 1], f32, tag="b1")
            nc.sync.dma_start(b1[:], b1_d[:])
            b2 = rp.tile([HID, 1], f32, tag="b2")
            nc.sync.dma_start(b2[:], b2_d[:])
            h1T_res = rp.tile([HID, SLAB], fp16, tag="h1T_res")
            red_sb = rp.tile([HID, SLAB], fp16, tag="red_sb")

            for k in range(8):
                nc.sync.dma_start(idx1r[16 * k:16 * (k + 1), :], idx1c[:])
            tc.strict_bb_all_engine_barrier()

            pid = nc.sync.partition_id()

            idx3 = idx1r.rearrange("p (s c) -> p s c", c=ic_st)
            dv3 = dvd.rearrange("p (s c) -> p s c", c=nch)
            inv3 = invd.rearrange("s (o d) -> s o d", o=1)

            def supertile_loop(lay, tabv, F, partials):
                p3 = partials.rearrange("f (s d) -> f s d", d=ST)
                gslots = gsz * P
                with tc.For_i(0, ns_run) as s:
                    idx_sb = ld.tile([P, ic_st], i16, tag=f"idx{lay}")
                    nc.sync.dma_start(idx_sb[:], idx3[:, s])
                    if lay == 2:
                        idx2t = ld.tile([P, ic_st], i16, tag="idx2t")
                        for g in range(ngath):
                            nc.vector.tensor_scalar(
                                idx2t[:, g * half:(g + 1) * half],
                                idx_sb[:, g * half:(g + 1) * half],
                                2, g, op0=mybir.AluOpType.mult,
                                op1=mybir.AluOpType.add)
                        idx_sb = idx2t
                    dvh = ld.tile([P, nch], fp16, tag=f"dvh{lay}")
                    nc.sync.dma_start(dvh[:], dv3[:, s])
                    dvf = ld.tile([P, nch], f32, tag=f"dvf{lay}")
                    nc.vector.tensor_copy(dvf[:], dvh[:])
                    inv1 = ld.tile([1, ST], fp16, tag=f"inv1_{lay}")
                    nc.sync.dma_start(inv1[:], inv3[s])
                    invb = ld.tile([F, ST], fp16, tag=f"invb{lay}")
                    nc.gpsimd.partition_broadcast(invb[:], inv1[:])
                    ms = []
                    for g in range(ngath):
                        m = gp.tile([P, gsz, P], fp16, tag=f"m{lay}_{g}")
                        nc.gpsimd.dma_gather(
                            m[:], tabv,
                            idx_sb[:, g * half:(g + 1) * half],
                            gslots, gslots, P)
                        ms.append(m)
                    zp = ps.tile([F, ST], f32, tag="zp")
                    for c in range(nch):
                        par4 = min(c // ch_par, 3)
                        colblk = (par4 * IN_CH) if lay == 1 else \
                            ((par4 & 1) * HID)
                        m = ms[c // gsz]
                        oh = ohp.tile([P, ST], fp16, tag=f"oh{lay}")
                        nc.vector.tensor_scalar(
                            oh[:], iota_sb[:], dvf[:, c:c + 1], None,
                            op0=mybir.AluOpType.is_equal)
                        nc.tensor.matmul(
                            zp[:], lhsT=m[:, c % gsz, colblk:colblk + F],
                            rhs=oh[:], start=(c == 0), stop=(c == nch - 1))
                    zsb = op_.tile([F, ST], fp16, tag=f"zsb{lay}")
                    nc.vector.tensor_tensor(zsb[:], zp[:], invb[:],
                                            op=mybir.AluOpType.mult)
                    nc.sync.dma_start(p3[:, s], zsb[:])

            xr3 = xg.rearrange("(t q) (k f) -> t (q k) f", q=P // 4, k=4)

            def dst_loop(lay, red, F, wl, wr, bb, hout_res):
                redv = red_sb[0:F, :]
                nc.sync.dma_start(
                    redv,
                    red.rearrange("f (c n) -> f c n", n=SLAB)[:, pid])
                r3 = redv.rearrange("f (t d) -> f t d", d=P)
                gr3 = grel_sb.rearrange("p (t o) -> p t o", o=1)
                o3 = (hout_res.rearrange("f (t d) -> f t d", d=P)
                      if hout_res is not None else None)
                h13 = h1d.rearrange("(t d) f -> t d f", d=P)
                h1r3 = h1T_res.rearrange("f (t d) -> f t d", d=P)
                with tc.For_i(0, nt_run) as t:
                    if lay == 1:
                        xr = ld.tile([P, IN_CH], fp16, tag="xr")
                        nc.sync.dma_start(xr[:], xr3[t])
                        xTp = ps3.tile([IN_CH, P], fp16, tag="xTp")
                        nc.tensor.transpose(xTp[:], xr[:], ident_sb[:])
                        xT = ld.tile([IN_CH, P], fp16, tag="xT")
                        nc.vector.tensor_copy(xT[:], xTp[:])
                    z2 = ps2.tile([HID, P], f32, tag="z2")
                    nc.tensor.matmul(z2[:], lhsT=wl[:], rhs=r3[:, t],
                                     start=True, stop=False)
                    if lay == 1:
                        nc.tensor.matmul(z2[:], lhsT=wr[:], rhs=xT[:],
                                         start=False, stop=True)
                    else:
                        nc.tensor.matmul(z2[:], lhsT=wr[:], rhs=h1r3[:, t],
                                         start=False, stop=True)
                    hT = op_.tile([HID, P], fp16, tag=f"hT{lay}")
                    nc.scalar.activation(hT[:], z2[:],
                                         mybir.ActivationFunctionType.Relu,
                                         bias=bb[:])
                    if o3 is not None:
                        nc.vector.tensor_copy(o3[:, t], hT[:])
                    tp = ps3.tile([P, HID], fp16, tag="tp")
                    nc.tensor.transpose(tp[:], hT[:],
                                        ident_sb[0:HID, 0:HID])
                    stg = stp.tile([P, HID], fp16, tag=f"stg{lay}")
                    nc.vector.tensor_copy(stg[:], tp[:])
                    if lay == 1:
                        nc.sync.dma_start(h13[t], stg[:])
                    else:
                        ohg = stp.tile([P, GT], fp16, tag="ohg")
                        nc.vector.tensor_scalar(
                            ohg[:], iota_sb[:, 0:GT], gr3[:, t],
                            None, op0=mybir.AluOpType.is_equal)
                        pp = ps3.tile([GT, HID], f32, tag="pp")
                        nc.tensor.matmul(pp[:], lhsT=ohg[:], rhs=stg[:],
                                         start=True, stop=True)
                        ppc = stp.tile([GT, HID], fp16, tag="ppc")
                        nc.vector.tensor_copy(ppc[:], pp[:])
                        nc.sync.dma_start(poolp[t], ppc[:])

            # ---- layer 1
            supertile_loop(1, xg[:], IN_CH, part1)
            nc.gpsimd.collective_compute(
                kind="AllReduce", op=mybir.AluOpType.add, replica_groups=GRP,
                ins=[part1[:]], outs=[red1[:]])
            dst_loop(1, red1, IN_CH, w1lT, w1rT, b1, h1T_res)

            # ---- layer 2
            h1v = h1d.rearrange("(r k) f -> r (k f)", k=2)
            supertile_loop(2, h1v, HID, part2)
            nc.gpsimd.collective_compute(
                kind="AllReduce", op=mybir.AluOpType.add, replica_groups=GRP,
                ins=[part2[:]], outs=[red2[:]])
            dst_loop(2, red2, HID, w2lT, w2rT, b2, None)

    nc.compile()
    return nc


_NC_CACHE = {}


def kernel(x, edge_index, batch, W1_l, b1, W1_r, W2_l, b2, W2_r, W_lin,
           b_lin, _timing=None):
    x = np.asarray(x, dtype=np.float32)
    batch_np = np.asarray(batch, dtype=np.int64)

    t0 = time.time()
    # speculatively build the expected-config NEFF while prep runs
    exp_key = (12, 2, 6, 3)
    th = None
    if exp_key not in _NC_CACHE:
        def _bg():
            try:
                _NC_CACHE[exp_key] = _build_nc(*exp_key)
            except Exception:
                pass
        th = threading.Thread(target=_bg)
        th.start()
    pp = _prep(edge_index, batch_np)
    t_prep = time.time() - t0

    nch, ngath, gsz = pp["nch"], pp["ngath"], pp["gsz"]
    t0 = time.time()
    if th is not None:
        th.join()
    key = (nch, ngath, gsz, pp["ch_par"])
    if key not in _NC_CACHE:
        _NC_CACHE[key] = _build_nc(nch, ngath, gsz, pp["ch_par"])
    nc = _NC_CACHE[key]
    t_build = time.time() - t0

    nb = pp["node_bounds"]
    iota_np = np.tile(np.arange(ST, dtype=np.float16), (P, 1))
    ident_np = np.eye(P, dtype=np.float16)
    com = dict(
        iota=iota_np, ident=ident_np, invd=pp["invrow"],
        w1lT=np.ascontiguousarray(np.asarray(W1_l).T).astype(np.float16),
        w1rT=np.ascontiguousarray(np.asarray(W1_r).T).astype(np.float16),
        w2lT=np.ascontiguousarray(np.asarray(W2_l).T).astype(np.float16),
        w2rT=np.ascontiguousarray(np.asarray(W2_r).T).astype(np.float16),
        b1c=np.asarray(b1, np.float32).reshape(HID, 1),
        b2c=np.asarray(b2, np.float32).reshape(HID, 1),
    )
    in_maps = []
    for c in range(N_CORES):
        lo, hi = nb[c], nb[c + 1]
        xs = np.zeros((SLAB, IN_CH), np.float16)
        xs[:hi - lo] = x[lo:hi].astype(np.float16)
        in_maps.append(dict(
            xg=np.ascontiguousarray(xs.reshape(SLAB * IN_CH // P, P)),
            idx1c=pp["idx1w"][c], dvd=pp["dv"][c], grel=pp["grel"][c],
            **com))

    t0 = time.time()
    res = run_bass_kernel_spmd(nc, in_maps, core_ids=list(range(N_CORES)))
    t_run = time.time() - t0

    # ---- host: combine pool windows + final linear
    t0 = time.time()
    pool = np.zeros((N_GRAPHS + GT, HID), np.float32)
    for c in range(N_CORES):
        ppart = res.results[c]["poolp"].astype(np.float32)   # [NT, GT, HID]
        gidx = pp["g0"][c][:, None] + np.arange(GT)[None, :]
        np.add.at(pool, gidx.reshape(-1), ppart.reshape(-1, HID))
    pool = pool[:N_GRAPHS]
    cnt = np.maximum(pp["gcnt"], 1).astype(np.float32)
    pooled = pool / cnt[:, None]
    out = (pooled @ np.asarray(W_lin, np.float32).T
           + np.asarray(b_lin, np.float32)).astype(np.float32)
    t_host = time.time() - t0

    if _timing is not None:
        _timing.update(dtA=t_run, dtB=0.0, prep=t_prep, build=t_build,
                       host=t_host)
    return out
